# revision 10
# baseline (speedup 1.0000x reference)
"""v3 Trainium2 Bass kernel for the 2-layer GAT on 8 NeuronCores.

dst-sharded graph parallelism, bf16 streams, per-block op batching.
v3 changes over v2:
- ad2[dst] expansion now rides the layer-2 gather: for every edge slot we
  gather BOTH the src pair-row and the dst pair-row of the table (the row
  already carries ad2 in col 17), killing the doffrow DMA, the rank-1 dx
  matmuls, the stT is_equal builds and the e2p matmuls.
- one dma_gather per dst-block (src idxs ++ dst idxs, num_idxs=2*SBB)
  instead of 19 per-subblock gathers.
- AllGather split into 4 uneven chunks issued as L1 completes their
  blocks, overlapping the collective with the L1 tail.
- log-softmax moved out of the per-block loop into one batched end pass
  (also removes the per-block activation-table thrash from Ln).
"""
import math
import numpy as np
import ml_dtypes

N = 50000
F = 64
H1, C1 = 8, 8
D1 = H1 * C1
NCLS = 16
E = 800000
NEG = 0.2
M = 8
NPC = N // M              # 6250
P = 128
NB = math.ceil(NPC / P)   # 49
PAIRS = NPC // 2          # 3125 table pair-rows per core
TROW = 128                # bf16 elems per table row (256B)
SUB = 64                  # bf16 elems per node sub-row (128B)
BPB = 7                   # subblocks per PSUM bank in g1 layout
NBK = 3                   # g1 banks per block

# chunked AllGather: block boundaries and pair-row ranges per chunk
CB = [0, 25, 37, 45, 49]
PS = [b * 64 for b in CB[:-1]] + [PAIRS]          # [0,1600,2368,2880,3125]
SZ = [PS[i + 1] - PS[i] for i in range(4)]        # rows per core per chunk
GB = [0]                                          # global chunk bases
for i in range(3):
    GB.append(GB[-1] + M * SZ[i])

bf = ml_dtypes.bfloat16
_LAST_NC = None
_LAST_INMAPS = None
_LAST_DIMS = None


def _pairrow(g):
    """Global node id -> row index in the chunk-ordered global table."""
    k, l = g // NPC, g % NPC
    p = l >> 1
    c = np.searchsorted(PS, p, side="right") - 1
    c = np.clip(c, 0, 3)
    szs = np.array(SZ)[c]
    return np.array(GB)[c] + k * szs + (p - np.array(PS)[c])


def _host_prep(x, src, dst):
    """Route edges to dst cores, group by 128-node dst block, pad to a
    uniform subblock count; build per-core upload arrays (vectorized)."""
    core = dst // NPC
    per_core_edges = []
    maxcnt = 0
    for k in range(M):
        m = np.nonzero(core == k)[0]
        s_k = src[m]
        dloc = dst[m] - k * NPC
        blk = dloc // P
        cnt = np.bincount(blk, minlength=NB)
        # dummy edges for pad rows of the partial last block
        rows_last = NPC - (NB - 1) * P
        cnt[NB - 1] += P - rows_last
        maxcnt = max(maxcnt, int(cnt.max()))
        per_core_edges.append((s_k, dloc, blk))
    SB2 = math.ceil(maxcnt / P)
    NSB = NB * SB2
    L = NSB * P

    per_core = []
    for k in range(M):
        s_k, dloc, blk = per_core_edges[k]
        # append dummy self-ish edges for pad rows of last block
        rows_last = NPC - (NB - 1) * P
        pad_r = np.arange(rows_last, P, dtype=np.int64)
        s_k = np.concatenate([s_k, np.zeros(len(pad_r), np.int64)])
        dloc = np.concatenate([dloc, (NB - 1) * P + pad_r])
        blk = np.concatenate([blk, np.full(len(pad_r), NB - 1, np.int64)])
        # slot assignment: sort by (blk, src), running position in block
        order = np.lexsort((s_k, blk))
        sb_ = blk[order]
        ss = s_k[order]
        dd = dloc[order]
        start = np.searchsorted(sb_, np.arange(NB))
        pos = np.arange(len(sb_)) - start[sb_]
        slot = sb_ * (SB2 * P) + pos
        srcs = np.zeros(L, np.int64)
        doff = np.full(L, -1.0, np.float32)
        valid = np.zeros(L, bool)
        srcs[slot] = ss
        doff[slot] = (dd - sb_ * P).astype(np.float32)
        valid[slot] = True
        dsts = np.where(valid,
                        (np.arange(L) // (SB2 * P)) * P + k * NPC
                        + np.maximum(doff, 0).astype(np.int64), 0)
        np.clip(dsts, 0, N - 1, out=dsts)
        # uploads
        xsd = np.concatenate([x[srcs], x[dsts]], axis=1).T  # [128, L]
        xsd = np.ascontiguousarray(xsd).astype(bf)
        dstoffT = np.ascontiguousarray(
            doff.reshape(NSB, P).T.astype(np.float32))      # [128, NSB]
        parityT = np.ascontiguousarray(
            (srcs & 1).reshape(NSB, P).T.astype(np.float32)).astype(bf)
        parityD = np.ascontiguousarray(
            (dsts & 1).reshape(NSB, P).T.astype(np.float32)).astype(bf)
        # gather idx: chunk-ordered global pair-rows for src and dst
        iv_s = _pairrow(srcs).reshape(NB, SB2, P)
        iv_d = _pairrow(dsts).reshape(NB, SB2, P)
        ivb = np.concatenate([iv_s, iv_d], axis=1).reshape(NB, 2 * SB2 * P)
        ii = np.arange(2 * SB2 * P)
        w = np.zeros((NB, 16, 2 * SB2 * 8), np.int16)
        w[:, ii % 16, ii // 16] = ivb.astype(np.int16)
        idx16 = np.tile(w, (1, 8, 1)).reshape(NB, 128, 2 * SB2 * 8)
        idx16 = np.ascontiguousarray(
            idx16.transpose(1, 0, 2).reshape(128, NB * 2 * SB2 * 8))
        per_core.append(dict(xsd=xsd, dstoffT=dstoffT, parityT=parityT,
                             parityD=parityD, idx16=idx16))
    return per_core, SB2, NSB, L


def _build_program(SB2, NSB, L, nq=4, dbg=None, rep=1):
    dbg = dbg or set()
    import concourse.bacc as bacc
    import concourse.mybir as mybir
    import concourse.tile as tile
    from concourse.library_config import mlp as mlp_lib

    fp32 = mybir.dt.float32
    bf16 = mybir.dt.bfloat16
    i16 = mybir.dt.int16
    AF = mybir.ActivationFunctionType
    OP = mybir.AluOpType
    AX = mybir.AxisListType

    nc = bacc.Bacc("TRN2", target_bir_lowering=False, debug=False,
                   num_devices=M, num_swdge_queues=nq)
    xsd_in = nc.dram_tensor("xsd", [P, L], bf16, kind="ExternalInput")
    dstoff_in = nc.dram_tensor("dstoffT", [P, NSB], fp32,
                               kind="ExternalInput")
    parity_in = nc.dram_tensor("parityT", [P, NSB], bf16,
                               kind="ExternalInput")
    parityd_in = nc.dram_tensor("parityD", [P, NSB], bf16,
                                kind="ExternalInput")
    idx_in = nc.dram_tensor("idx16", [P, NSB * 16], i16,
                            kind="ExternalInput")
    WA_in = nc.dram_tensor("WAbf", [P, 72], bf16, kind="ExternalInput")
    W2X_in = nc.dram_tensor("W2Xbf", [D1, SUB], bf16, kind="ExternalInput")
    IOTA_in = nc.dram_tensor("IOTAbf", [P, P], bf16, kind="ExternalInput")
    ID_in = nc.dram_tensor("IDENTbf", [P, P], bf16, kind="ExternalInput")
    b1_in = nc.dram_tensor("b1", [1, D1], fp32, kind="ExternalInput")
    b2_in = nc.dram_tensor("b2", [1, NCLS], fp32, kind="ExternalInput")
    out_ext = nc.dram_tensor("out", [NPC, NCLS], fp32, kind="ExternalOutput")

    SBB = SB2 * P   # slots per block

    with tile.TileContext(nc) as tc:
        nc.gpsimd.load_library(mlp_lib)
        with (
            tc.tile_pool(name="const", bufs=1) as cp,
            tc.tile_pool(name="dram", bufs=1, space="DRAM") as dr,
        ):
            WAbf = cp.tile([P, 72], bf16)
            nc.sync.dma_start(out=WAbf[:], in_=WA_in.ap())
            W2Xbf = cp.tile([D1, SUB], bf16)
            nc.sync.dma_start(out=W2Xbf[:], in_=W2X_in.ap())
            IOTAbf = cp.tile([P, P], bf16)
            nc.sync.dma_start(out=IOTAbf[:], in_=IOTA_in.ap())
            IDENTbf = cp.tile([P, P], bf16)
            nc.sync.dma_start(out=IDENTbf[:], in_=ID_in.ap())
            b1rep = cp.tile([P, D1], fp32)
            nc.sync.dma_start(out=b1rep[:],
                              in_=b1_in.ap().to_broadcast((P, D1)))
            b2rep = cp.tile([P, NCLS], fp32)
            nc.sync.dma_start(out=b2rep[:],
                              in_=b2_in.ap().to_broadcast((P, NCLS)))
            dstoffT = cp.tile([P, NSB], fp32)
            nc.sync.dma_start(out=dstoffT[:], in_=dstoff_in.ap())
            parityT = cp.tile([P, NSB], bf16)
            nc.sync.dma_start(out=parityT[:], in_=parity_in.ap())
            parityD = cp.tile([P, NSB], bf16)
            nc.sync.dma_start(out=parityD[:], in_=parityd_in.ap())
            idx16 = cp.tile([P, NSB * 16], i16)
            nc.sync.dma_start(out=idx16[:], in_=idx_in.ap())
            outa17 = cp.tile([P, NB * 17], fp32)
            outacc = cp.tile([P, NB * NCLS], fp32)

            T2p = [dr.tile([SZ[c], TROW], bf16, tag=f"t2p{c}",
                           name=f"T2p{c}") for c in range(4)]
            serdum = cp.tile([P, 1], fp32)

            for _rep in range(rep):
                if _rep > 0:
                    # serialize reps so rep-slope timing matches single-shot:
                    # rep r+1's L1 matmuls read WAbf, which is rewritten here
                    # with a zero add that depends on rep r's final outacc.
                    nc.vector.tensor_scalar(
                        out=serdum[:], in0=outacc[:, 0:1], scalar1=0.0,
                        scalar2=None, op0=OP.mult)
                    nc.vector.tensor_tensor(
                        out=WAbf[:], in0=WAbf[:],
                        in1=serdum[:].to_broadcast((P, 72)), op=OP.add)
                # local (non-Shared) so 4 chunked collectives may write it
                T2tbl = dr.tile([M * PAIRS, TROW], bf16,
                                tag=f"t2tbl{_rep}", name=f"T2tbl{_rep}")
                # ================= layer 1 =================
                with (
                    tc.tile_pool(name="sb1", bufs=2) as sb1,
                    tc.tile_pool(name="ps1", bufs=1, space="PSUM") as ps1,
                ):
                  for b in ([] if "skip_l1" in dbg else range(NB)):
                    rows = min(P, NPC - b * P)
                    xsd = sb1.tile([P, SBB], bf16, tag="xsd", bufs=2)
                    nc.sync.dma_start(
                        out=xsd[:], in_=xsd_in.ap()[:, b * SBB:(b + 1) * SBB])
                    g1 = [ps1.tile([P, BPB * 72], fp32, tag=f"g1{t}",
                                   name=f"g1{t}",
                                   bufs=(2 if t < 2 else 1))
                          for t in range(NBK)]
                    for s in range(SB2):
                        t, j = divmod(s, BPB)
                        nc.tensor.matmul(
                            out=g1[t][:, j * 72:(j + 1) * 72],
                            lhsT=xsd[:, s * P:(s + 1) * P], rhs=WAbf[:],
                            start=True, stop=True)
                    # batched leakyrelu (per bank tile) + exp (one op)
                    eL = sb1.tile([P, SB2 * H1], bf16, tag="eL")
                    for t in range(NBK):
                        ns = min(BPB, SB2 - t * BPB)
                        nc.scalar.activation(
                            eL[:, t * BPB * H1:(t * BPB + ns) * H1]
                            .rearrange("p (s h) -> p s h", s=ns),
                            g1[t][:, 0:ns * 72]
                            .rearrange("p (s c) -> p s c", s=ns)
                            [:, :, 64:72],
                            AF.Prelu, alpha=NEG)
                    ex = sb1.tile([P, SB2 * H1], bf16, tag="ex")
                    nc.scalar.activation(ex[:], eL[:], AF.Exp)
                    # m_t: [ex*h | ex] per subblock
                    m_t = sb1.tile([P, SB2 * 72], bf16, tag="mt")
                    for t in range(NBK):
                        ns = min(BPB, SB2 - t * BPB)
                        nc.vector.tensor_tensor(
                            out=m_t[:, t * BPB * 72:(t * BPB + ns) * 72]
                            .rearrange("p (s c) -> p s c", s=ns)
                            [:, :, 0:64]
                            .rearrange("p s (h c) -> p s h c", h=H1),
                            in0=g1[t][:, 0:ns * 72]
                            .rearrange("p (s c) -> p s c", s=ns)
                            [:, :, 0:64]
                            .rearrange("p s (h c) -> p s h c", h=H1),
                            in1=ex[:, t * BPB * H1:(t * BPB + ns) * H1]
                            .rearrange("p (s h) -> p s h", s=ns)
                            [:, :, :, None].to_broadcast((P, ns, H1, C1)),
                            op=OP.mult)
                    nc.vector.tensor_copy(
                        m_t[:].rearrange("p (s c) -> p s c", s=SB2)
                        [:, :, 64:72],
                        ex[:].rearrange("p (s h) -> p s h", s=SB2))
                    # s_t one-hots + aggregation matmuls
                    agg = ps1.tile([P, 72], fp32, tag="agg", bufs=1)
                    for s in range(SB2):
                        sbk = b * SB2 + s
                        st = sb1.tile([P, P], bf16, tag="st", bufs=6)
                        nc.vector.tensor_scalar(
                            out=st[:], in0=IOTAbf[:],
                            scalar1=dstoffT[:, sbk:sbk + 1], scalar2=None,
                            op0=OP.is_equal)
                        nc.tensor.matmul(
                            out=agg[:], lhsT=st[:],
                            rhs=m_t[:, s * 72:(s + 1) * 72],
                            start=(s == 0), stop=(s == SB2 - 1))
                    # finalize block
                    aggc = sb1.tile([P, 72], fp32, tag="aggc")
                    nc.vector.tensor_copy(aggc[:], agg[:])
                    zinv = sb1.tile([P, H1], fp32, tag="zinv")
                    nc.vector.reciprocal(out=zinv[:], in_=aggc[:, 64:72])
                    y = sb1.tile([P, D1], fp32, tag="y")
                    nc.vector.tensor_tensor(
                        out=y[:].rearrange("p (h c) -> p h c", h=H1),
                        in0=aggc[:, 0:64].rearrange("p (h c) -> p h c", h=H1),
                        in1=zinv[:, :, None].to_broadcast((P, H1, C1)),
                        op=OP.mult)
                    t0 = sb1.tile([P, D1], fp32, tag="t0")
                    nc.vector.tensor_add(out=t0[:], in0=y[:], in1=b1rep[:])
                    tm = sb1.tile([P, D1], fp32, tag="tm")
                    nc.vector.tensor_scalar_min(out=tm[:], in0=t0[:],
                                                scalar1=0.0)
                    u = sb1.tile([P, D1], fp32, tag="u")
                    nc.scalar.activation(u[:], tm[:], AF.Exp)
                    r = sb1.tile([P, D1], fp32, tag="r")
                    nc.scalar.activation(r[:], t0[:], AF.Relu)
                    v = sb1.tile([P, D1], fp32, tag="v")
                    nc.vector.tensor_scalar(out=v[:], in0=u[:], scalar1=1.0,
                                            scalar2=1.0, op0=OP.min,
                                            op1=OP.subtract)
                    h1bf = sb1.tile([P, D1], bf16, tag="h1")
                    nc.vector.tensor_add(out=h1bf[:], in0=r[:], in1=v[:])
                    h1Tp = ps1.tile([D1, P], bf16, tag="tpb", bufs=1)
                    nc.tensor.transpose(out=h1Tp[:], in_=h1bf[:],
                                        identity=IDENTbf[:])
                    h1Ts = sb1.tile([D1, P], bf16, tag="h1T")
                    nc.vector.tensor_copy(h1Ts[:], h1Tp[:])
                    t2p = ps1.tile([P, SUB], fp32, tag="tpf", bufs=1)
                    nc.tensor.matmul(out=t2p[:], lhsT=h1Ts[:], rhs=W2Xbf[:],
                                     start=True, stop=True)
                    t2sb = sb1.tile([P, SUB], bf16, tag="t2")
                    nc.vector.tensor_copy(t2sb[:], t2p[:])
                    c = next(i for i in range(4) if CB[i] <= b < CB[i + 1])
                    rbase = b * 64 - PS[c]
                    nc.sync.dma_start(
                        out=T2p[c][rbase:rbase + rows // 2 + (rows % 2), :]
                        .rearrange("r (q c) -> (r q) c", q=2)[0:rows],
                        in_=t2sb[:rows])

                # ============ chunked exchange ============
                if "skip_ag" not in dbg and "skip_l1" not in dbg:
                    for c in range(4):
                        nc.gpsimd.collective_compute(
                            "AllGather", mybir.AluOpType.bypass,
                            replica_groups=[list(range(M))],
                            ins=[T2p[c][:]],
                            outs=[T2tbl[GB[c]:GB[c] + M * SZ[c], :]])

                # ================= layer 2 =================
                with (
                    tc.tile_pool(name="sb2", bufs=2) as sb2,
                    tc.tile_pool(name="ps2", bufs=1, space="PSUM") as ps2,
                ):
                  GSUB = 7   # subblocks per gather call (ring: <=1024 descs)
                  for b in ([] if "skip_l2" in dbg else range(NB)):
                    g2 = sb2.tile([P, 2 * SBB], bf16, tag="g2", bufs=2)
                    ib = b * SB2 * 16
                    for gs in range(0, 2 * SB2, GSUB):
                        ng = min(GSUB, 2 * SB2 - gs)
                        nc.gpsimd.dma_gather(
                            out_ap=g2[:, gs * P:(gs + ng) * P]
                            .rearrange("p (g d) -> p g d", g=ng),
                            in_ap=T2tbl[:],
                            idxs_ap=idx16[:, ib + gs * 8:ib + (gs + ng) * 8],
                            num_idxs=ng * P, num_idxs_reg=ng * P,
                            elem_size=TROW, queue_num=(b + gs) % nq)
                    g2v = g2[:, 0:SBB].rearrange("p (s d) -> p s d", s=SB2)
                    g2d = g2[:, SBB:2 * SBB].rearrange("p (s d) -> p s d",
                                                       s=SB2)
                    # parity select of src [h2(16)|as2(1)] sub-rows
                    sel = sb2.tile([P, SB2 * 17], bf16, tag="sel")
                    selv = sel[:].rearrange("p (s c) -> p s c", s=SB2)
                    dif = sb2.tile([P, SB2 * 17], bf16, tag="dif")
                    difv = dif[:].rearrange("p (s c) -> p s c", s=SB2)
                    nc.vector.tensor_tensor(
                        out=difv, in0=g2v[:, :, SUB:SUB + 17],
                        in1=g2v[:, :, 0:17], op=OP.subtract)
                    nc.vector.tensor_tensor(
                        out=difv, in0=difv,
                        in1=parityT[:, b * SB2:(b + 1) * SB2]
                        [:, :, None].to_broadcast((P, SB2, 17)),
                        op=OP.mult)
                    nc.vector.tensor_tensor(
                        out=selv, in0=difv, in1=g2v[:, :, 0:17], op=OP.add)
                    # parity select of dst ad2 (col 17 of each sub-row)
                    difd = sb2.tile([P, SB2], bf16, tag="difd")
                    nc.vector.tensor_tensor(
                        out=difd[:], in0=g2d[:, :, SUB + 17],
                        in1=g2d[:, :, 17], op=OP.subtract)
                    nc.vector.tensor_tensor(
                        out=difd[:], in0=difd[:],
                        in1=parityD[:, b * SB2:(b + 1) * SB2], op=OP.mult)
                    ad2e = sb2.tile([P, SB2], bf16, tag="ad2e")
                    nc.vector.tensor_tensor(
                        out=ad2e[:], in0=difd[:], in1=g2d[:, :, 17],
                        op=OP.add)
                    # e2 = as2[src] + ad2[dst]; exp(lrelu(.))
                    e2s = sb2.tile([P, SB2], bf16, tag="e2s")
                    nc.vector.tensor_tensor(
                        out=e2s[:], in0=ad2e[:], in1=selv[:, :, 16],
                        op=OP.add)
                    eL2 = sb2.tile([P, SB2], bf16, tag="eL2")
                    nc.scalar.activation(eL2[:], e2s[:], AF.Prelu, alpha=NEG)
                    ex2 = sb2.tile([P, SB2], bf16, tag="ex2")
                    nc.scalar.activation(ex2[:], eL2[:], AF.Exp)
                    # m2 = [ex2 | ex2*h2]
                    m2 = sb2.tile([P, SB2 * 17], bf16, tag="m2")
                    m2v = m2[:].rearrange("p (s c) -> p s c", s=SB2)
                    nc.vector.tensor_tensor(
                        out=m2v[:, :, 1:17], in0=selv[:, :, 0:16],
                        in1=ex2[:, :, None].to_broadcast((P, SB2, 16)),
                        op=OP.mult)
                    nc.vector.tensor_copy(m2v[:, :, 0:1], ex2[:, :, None])
                    # aggregation
                    agg2 = ps2.tile([P, 17], fp32, tag="agg2", bufs=2)
                    for s in range(SB2):
                        sbk = b * SB2 + s
                        st2 = sb2.tile([P, P], bf16, tag="st2", bufs=6)
                        nc.vector.tensor_scalar(
                            out=st2[:], in0=IOTAbf[:],
                            scalar1=dstoffT[:, sbk:sbk + 1], scalar2=None,
                            op0=OP.is_equal)
                        nc.tensor.matmul(
                            out=agg2[:], lhsT=st2[:],
                            rhs=m2[:, s * 17:(s + 1) * 17],
                            start=(s == 0), stop=(s == SB2 - 1))
                    # drain psum; rest of finalize is batched at the end
                    nc.vector.tensor_copy(outa17[:, b * 17:(b + 1) * 17],
                                          agg2[:])
                  # ======== batched finalize: log_softmax(num/Z + b2) ======
                  if "skip_l2" not in dbg:
                    o17 = outa17[:].rearrange("p (b c) -> p b c", c=17)
                    zw = sb2.tile([P, NB], fp32, tag="zw")
                    nc.vector.reciprocal(out=zw[:], in_=o17[:, :, 0])
                    lgw = sb2.tile([P, NB * NCLS], fp32, tag="lgw")
                    lgv = lgw[:].rearrange("p (b c) -> p b c", c=NCLS)
                    nc.vector.tensor_tensor(
                        out=lgv, in0=o17[:, :, 1:17],
                        in1=zw[:, :, None].to_broadcast((P, NB, NCLS)),
                        op=OP.mult)
                    nc.vector.tensor_tensor(
                        out=lgv, in0=lgv,
                        in1=b2rep[:].rearrange("p (x c) -> p x c", x=1)
                        .to_broadcast((P, NB, NCLS)),
                        op=OP.add)
                    et = sb2.tile([P, NB * NCLS], fp32, tag="et")
                    nc.scalar.activation(et[:], lgw[:], AF.Exp)
                    sew = sb2.tile([P, NB], fp32, tag="sew")
                    nc.vector.tensor_reduce(
                        out=sew[:],
                        in_=et[:].rearrange("p (b c) -> p b c", c=NCLS),
                        axis=AX.X, op=OP.add)
                    lsew = sb2.tile([P, NB], fp32, tag="lsew")
                    nc.scalar.activation(lsew[:], sew[:], AF.Ln)
                    nc.vector.tensor_tensor(
                        out=outacc[:].rearrange("p (b c) -> p b c", c=NCLS),
                        in0=lgv,
                        in1=lsew[:, :, None].to_broadcast((P, NB, NCLS)),
                        op=OP.subtract)
                # output: two DMAs (full blocks + partial last block)
                rows_last = NPC - (NB - 1) * P
                nc.sync.dma_start(
                    out=out_ext.ap()[0:(NB - 1) * P, :]
                    .rearrange("(b p) c -> p b c", p=P),
                    in_=outacc[:, 0:(NB - 1) * NCLS]
                    .rearrange("p (b c) -> p b c", c=NCLS))
                nc.sync.dma_start(
                    out=out_ext.ap()[(NB - 1) * P:NPC, :],
                    in_=outacc[:rows_last, (NB - 1) * NCLS:NB * NCLS])
    nc.compile()
    return nc


def _make_consts(W1, a_src1, a_dst1, W2, a_src2, a_dst2):
    W1r = W1.reshape(F, H1, C1)
    As_x = np.einsum("fhc,hc->fh", W1r, a_src1)
    Ad_x = np.einsum("fhc,hc->fh", W1r, a_dst1)
    WA = np.zeros((P, 72), np.float32)
    WA[0:F, 0:64] = W1
    WA[0:F, 64:72] = As_x
    WA[F:2 * F, 64:72] = Ad_x
    W2X = np.zeros((D1, 64), np.float32)
    W2X[:, 0:16] = W2
    W2X[:, 16] = W2 @ a_src2[0]
    W2X[:, 17] = W2 @ a_dst2[0]
    IOTA = np.broadcast_to(np.arange(P, dtype=np.float32), (P, P))
    IDENT = np.eye(P, dtype=np.float32)
    return (WA.astype(bf), W2X.astype(bf),
            np.ascontiguousarray(IOTA).astype(bf), IDENT.astype(bf))


def _prepare(x, edge_index, W1, a_src1, a_dst1, b1, W2, a_src2, a_dst2, b2):
    loop = np.arange(N, dtype=np.int64)
    src = np.concatenate([edge_index[0].astype(np.int64), loop])
    dst = np.concatenate([edge_index[1].astype(np.int64), loop])
    per_core, SB2, NSB, L = _host_prep(
        np.asarray(x, np.float32), src, dst)
    WAbf, W2Xbf, IOTAbf, IDENTbf = _make_consts(
        np.asarray(W1, np.float32), np.asarray(a_src1, np.float32),
        np.asarray(a_dst1, np.float32), np.asarray(W2, np.float32),
        np.asarray(a_src2, np.float32), np.asarray(a_dst2, np.float32))
    in_maps = []
    for k in range(M):
        d = per_core[k]
        in_maps.append({
            "xsd": d["xsd"], "dstoffT": d["dstoffT"],
            "parityT": d["parityT"], "parityD": d["parityD"],
            "idx16": d["idx16"],
            "WAbf": WAbf, "W2Xbf": W2Xbf, "IOTAbf": IOTAbf,
            "IDENTbf": IDENTbf,
            "b1": np.asarray(b1, np.float32)[None, :],
            "b2": np.asarray(b2, np.float32)[None, :],
        })
    return in_maps, (SB2, NSB, L)


def kernel(x, edge_index, W1, a_src1, a_dst1, b1, W2, a_src2, a_dst2, b2):
    in_maps, dims = _prepare(x, np.asarray(edge_index), W1, a_src1, a_dst1,
                             b1, W2, a_src2, a_dst2, b2)
    nc = _build_program(*dims)
    from concourse.bass_utils import run_bass_kernel_spmd
    global _LAST_NC, _LAST_INMAPS, _LAST_DIMS
    _LAST_NC, _LAST_INMAPS = nc, in_maps
    _LAST_DIMS = dims
    res = run_bass_kernel_spmd(nc, in_maps, list(range(M))).results
    out = np.concatenate([res[k]["out"] for k in range(M)], axis=0)
    return out.astype(np.float32)
